# revision 4
# baseline (speedup 1.0000x reference)
"""MHA kernel v2 for TRN2 (per-core SPMD program) + host prep.

out = X + MHA(RMSNorm(X)) with Q=K=V=(RMSNorm(X)@Wq.T+b), rope, causal
softmax, Wo. Sharding: batch(2) x head-groups(4) over 8 cores; each core
computes a partial (its 4 heads through Wo) in f16; host sums partials.

v2 vs baseline:
- chunk-pipelined prologue: XT DMA'd c-major so Q-proj starts ~7us in
- ssq + softmax denominators via col-tiled M=1 matmuls (4 concurrent)
- one batched reciprocal_approx_fast per q-chunk instead of 16 [1,512]
  DVE reciprocals
- V^T via DMA-engine transpose (4 DMAs) instead of 64 PE transposes
- broadcasts on GPSIMD (partition_broadcast) instead of PE+copy
- attention q-chunk loop with Wo(qc-1) interleaved to keep PE dense
- exp -> bf16 attention tiles; f16 output partials
"""
import math
import itertools
import numpy as np
import ml_dtypes
from contextlib import ExitStack

import concourse.bass as bass
import concourse.mybir as mybir
import concourse.tile as tile

F32 = mybir.dt.float32
F32R = mybir.dt.float32r
F16 = mybir.dt.float16
BF16 = mybir.dt.bfloat16

EPS = float(np.finfo(np.float32).eps)
ROPE_BASE = 10000.0

_ctr = itertools.count()


def legalize_sync_waits(nc, max_waits=1):
    """Walrus accepts at most one sync-wait per instruction; hoist excess
    waits onto same-engine NOPs inserted just before."""
    n_fixed = 0
    for f in nc.m.functions:
        for bb in f.blocks:
            insts = bb.instructions
            out = []
            dirty = False
            for inst in insts:
                si = getattr(inst, "sync_info", None)
                if si is not None and si.on_wait and len(si.on_wait) > max_waits:
                    waits = list(si.on_wait)
                    for w in waits[:-max_waits]:
                        nop = mybir.InstNoOp(
                            name=f"I-syncfix-{next(_ctr)}", engine=inst.engine
                        )
                        nop.sync_info = mybir.SyncInfo(on_wait=[w], on_update=[])
                        nc.register_instruction(nop, overwrite=True)
                        out.append(nop)
                    inst.sync_info = mybir.SyncInfo(
                        on_wait=waits[-max_waits:], on_update=list(si.on_update or [])
                    )
                    dirty = True
                    n_fixed += 1
                out.append(inst)
            if dirty:
                bb.instructions = out
    return n_fixed


def build_core(S=2048, D=2048, NHL=4, DK=128, SHIFT=10.0, debug=False):
    assert S % 512 == 0 and D % 128 == 0 and DK == 128
    SK = S // 512     # 512-wide seq chunks
    KT = D // 128     # contraction tiles for projections
    ST = S // 128     # 128-wide seq tiles
    ML = NHL * DK     # local model width
    NC_ = D // 512    # output column chunks

    nc = bass.Bass("TRN2", num_devices=8)
    # host-pre-shuffled layouts so every DMA moves big contiguous rows:
    # XTS[p, c, k, s] (16KB/partition per chunk), WQS[p, k*ML], WOS[p, h*D]
    if debug:
        dQTH = nc.dram_tensor("DQTH", [DK, S], F16, kind="ExternalOutput")
        dQR = nc.dram_tensor("DQR", [DK, S], F16, kind="ExternalOutput")
        dVV = nc.dram_tensor("DVV", [128, ST, DK], F16, kind="ExternalOutput")
        dOT = nc.dram_tensor("DOT", [DK, S], F16, kind="ExternalOutput")
        dRBC = nc.dram_tensor("DRBC", [128, S], F16, kind="ExternalOutput")
        dDN = nc.dram_tensor("DDN", [128, 4, 512], F32, kind="ExternalOutput")
        dRDB = nc.dram_tensor("DRDB", [128, 4, 512], F32, kind="ExternalOutput")
        dOPS = nc.dram_tensor("DOPS", [DK, S], F32, kind="ExternalOutput")
        dATQ = nc.dram_tensor("DATQ", [128, 16, 512], BF16, kind="ExternalOutput")
    dXT = nc.dram_tensor("XT", [128, SK, KT, 512], F16, kind="ExternalInput")
    dWQT = nc.dram_tensor("WQT", [128, NHL, KT, 128], F16, kind="ExternalInput")
    dWOT = nc.dram_tensor("WOT", [128, NHL, D], F16, kind="ExternalInput")
    dQB = nc.dram_tensor("QB", [128, NHL], F32, kind="ExternalInput")
    dCOSA = nc.dram_tensor("COSA", [DK, S], F16, kind="ExternalInput")
    dSINA = nc.dram_tensor("SINA", [DK, S], F16, kind="ExternalInput")
    dMASKD = nc.dram_tensor("MASKD", [128, 4, 512], BF16, kind="ExternalInput")
    dOUT = nc.dram_tensor("OUTP", [S, D], F16, kind="ExternalOutput")

    with tile.TileContext(nc) as tc, ExitStack() as ctx:
        pp = ctx.enter_context(tc.tile_pool(name="pp", bufs=1))

        # ---- persistent constants + per-head tensors ---------------------
        cosa = pp.tile([DK, S], F16, name="cosa")
        sina = pp.tile([DK, S], F16, name="sina")
        maskd = pp.tile([128, 4, 512], BF16, name="maskd")
        qb = pp.tile([128, NHL], F32, name="qb")
        shift_t = pp.tile([128, 1], F32, name="shift_t")
        ones_bf = pp.tile([128, 1], BF16, name="ones_bf")
        ones_f16 = pp.tile([128, 1], F16, name="ones_f16")
        onesr_bf = pp.tile([128, 128], BF16, name="onesr_bf")
        onesr_f16 = pp.tile([128, 128], F16, name="onesr_f16")
        eps_t = pp.tile([1, 1], F32, name="eps_t")
        qr = [pp.tile([DK, S], F16, name=f"qr{h}") for h in range(NHL)]
        atq0 = [pp.tile([128, NHL, 512], BF16, name=f"at{pt}")
                for pt in range(4)]
        vv = [pp.tile([128, ST, DK], F16, name=f"vv{h}") for h in range(NHL)]
        ot = [pp.tile([DK, S], F16, name=f"ot{h}") for h in range(NHL)]

        # consts on the gpsimd swdge queue (not latency-critical); weights
        # and activations on the two hwdge queues
        nc.gpsimd.dma_start(out=cosa, in_=dCOSA[:, :])
        nc.gpsimd.dma_start(out=sina, in_=dSINA[:, :])
        nc.gpsimd.dma_start(out=maskd, in_=dMASKD[:, :, :])
        nc.gpsimd.dma_start(out=qb, in_=dQB[:, :])
        nc.vector.memset(shift_t, -SHIFT)
        nc.vector.memset(ones_bf, 1.0)
        nc.vector.memset(ones_f16, 1.0)
        nc.vector.memset(onesr_bf, 1.0)
        nc.vector.memset(onesr_f16, 1.0)
        nc.vector.memset(eps_t, EPS)

        with tc.tile_pool(name="pxw", bufs=1) as pxw, \
                tc.tile_pool(name="psP", bufs=1, space="PSUM") as ps:
            xt = pxw.tile([128, SK, KT, 512], F16, name="xt")
            wq = pxw.tile([128, NHL, KT, 128], F16, name="wq")
            qth = [pxw.tile([DK, S], F16, name=f"qth{h}") for h in range(NHL)]
            Rbc = pxw.tile([128, S], F16, name="Rbc")
            rrow = pxw.tile([1, S], F32, name="rrow")

            # weights first on the scalar queue (host pre-shuffled, 16KB
            # rows); head 0 separately so qproj(c0, mt0) can start early
            nc.scalar.dma_start(out=wq[:, 0, :, :], in_=dWQT[:, 0, :, :])
            nc.scalar.dma_start(out=wq[:, 1:NHL, :, :], in_=dWQT[:, 1:NHL, :, :])

            # XT chunk-major so chunk c completes early; alternate queues
            for c in range(SK):
                eng = nc.sync if c % 2 == 0 else nc.scalar
                eng.dma_start(out=xt[:, c, :, :], in_=dXT[:, c, :, :])

            # ---- PE warm-up: dummy matmuls while the first DMAs land ----
            warm_ps = ps.tile([128, 512], F32, name="warm", tag="A", bufs=2)
            for i in range(40):
                nc.tensor.matmul(warm_ps[:, 0:128], onesr_f16, onesr_f16,
                                 start=True, stop=True)

            # ---- ssq via col-tiled M=1 ones-matmuls ----------------------
            ssq_ps = ps.tile([128, 512], F32, name="ssq", tag="dn", bufs=1)
            for c in range(SK):
                for k in range(KT):
                    sq = pxw.tile([128, 512], F16, name=f"sq{c}_{k}",
                                  tag="sqk", bufs=2)
                    nc.vector.tensor_mul(sq, xt[:, c, k, :], xt[:, c, k, :])
                    nc.tensor.matmul(ssq_ps[32 * c:32 * c + 1, :], ones_f16, sq,
                                     start=(k == 0), stop=(k == KT - 1),
                                     tile_position=(0, 32 * c))
            for c in range(SK):
                sl = slice(c * 512, (c + 1) * 512)
                # r = (ssq/D + eps)^-1/2 via exp(-0.5*ln(x)) (one ACT table
                # set shared with the attention exps), then DMA row-broadcast
                nc.scalar.activation(rrow[:, sl], ssq_ps[32 * c:32 * c + 1, :],
                                     mybir.ActivationFunctionType.Ln,
                                     bias=eps_t, scale=1.0 / D)
                rf16 = pxw.tile([1, 512], F16, name=f"rf{c}", tag="rf16",
                                bufs=2)
                nc.scalar.activation(rf16, rrow[:, sl],
                                     mybir.ActivationFunctionType.Exp,
                                     scale=-0.5)
                rb_ps = ps.tile([128, 512], F32, name=f"rbps{c}", tag="B",
                                bufs=2)
                nc.tensor.matmul(rb_ps, onesr_f16[0:1, :], rf16,
                                 start=True, stop=True)
                nc.vector.tensor_copy(Rbc[:, sl], rb_ps)

            # ---- Q projection + per-chunk rope, c-major -----------------
            hw = DK // 2

            def qproj(c, mt):
                sl = slice(c * 512, (c + 1) * 512)
                q_ps = ps.tile([128, 512], F32, name=f"qps{c}_{mt}",
                               tag="A", bufs=2)
                for k in range(KT):
                    nc.tensor.matmul(q_ps,
                                     wq[:, mt, k, :],
                                     xt[:, c, k, :],
                                     start=(k == 0), stop=(k == KT - 1))
                tmp = pxw.tile([128, 512], F32, name=f"tq{c}_{mt}",
                               tag="tq", bufs=2)
                nc.vector.tensor_mul(tmp, q_ps, Rbc[:, sl])
                nc.scalar.activation(qth[mt][:, sl], tmp,
                                     mybir.ActivationFunctionType.Identity,
                                     bias=qb[:, mt:mt + 1])

            def rope(c, mt):
                # qr = qth*cosa + rot_half(qth)*sina  (sign folded in SINA)
                sl = slice(c * 512, (c + 1) * 512)
                sh = pxw.tile([DK, 512], F16, name=f"sh{mt}_{c}", tag="sh",
                              bufs=2)
                m1 = pxw.tile([DK, 512], F16, name=f"m1{mt}_{c}", tag="m1",
                              bufs=2)
                nc.vector.tensor_copy(sh[0:hw, :], qth[mt][hw:DK, sl])
                nc.vector.tensor_copy(sh[hw:DK, :], qth[mt][0:hw, sl])
                nc.vector.tensor_mul(m1, qth[mt][:, sl], cosa[:, sl])
                nc.vector.tensor_mul(sh, sh, sina[:, sl])
                nc.vector.tensor_add(qr[mt][:, sl], m1, sh)

            def scores_pt(qc, pt, s_pool, s_bufs, atq_t):
                off = pt * 128 - qc * 512
                lo = max(0, off)
                n = 512 - lo
                for hp in range(2):
                    s2 = s_pool.tile([128, 2, 512], F32,
                                     name=f"s2_{qc}_{pt}_{hp}", tag="S",
                                     bufs=s_bufs)
                    for hh in range(2):
                        h = hp * 2 + hh
                        nc.tensor.matmul(
                            s2[:, hh, 0:n],
                            qr[h][:, pt * 128:(pt + 1) * 128],
                            qr[h][:, qc * 512 + lo:(qc + 1) * 512],
                            start=True, stop=True)
                    nc.scalar.activation(
                        atq_t[:, 2 * hp:2 * hp + 2, lo:512],
                        s2[:, :, 0:n],
                        mybir.ActivationFunctionType.Exp, bias=shift_t)
                if off >= 0:
                    v = off // 128
                    nc.vector.tensor_mul(
                        atq_t, atq_t,
                        maskd[:, v:v + 1, :].to_broadcast((128, NHL, 512)))

            for c in range(SK):
                for mt in range(NHL):
                    qproj(c, mt)
                    rope(c, mt)
                if c == 0:
                    # qc=0 attention scores only touch chunk 0 of qr
                    for pt in range(4):
                        lo0 = 128 * (pt % 4)
                        if lo0 > 0:
                            nc.gpsimd.memset(atq0[pt][:, :, 0:lo0], 0.0)
                    for pt in range(4):
                        scores_pt(0, pt, ps, 1, atq0[pt])
            for mt in range(NHL):
                # V^T via DMA transpose (whole head in one shot)
                nc.sync.dma_start_transpose(vv[mt], qth[mt])
            if debug:
                nc.gpsimd.dma_start(out=dQTH[:, :], in_=qth[0])
                nc.gpsimd.dma_start(out=dRBC[:, :], in_=Rbc)

        # ---- attention + interleaved Wo ---------------------------------
        with tc.tile_pool(name="pback", bufs=1) as pback, \
                tc.tile_pool(name="psA", bufs=1, space="PSUM") as ps:
            wo = pback.tile([128, NHL, D], F16, name="wo")
            nc.scalar.dma_start(out=wo, in_=dWOT[:, :, :])
            # one tile per key-tile holding all 4 heads' attention weights
            # (pt 0-3 live in the persistent pool: written during qproj)
            atq = atq0 + [pback.tile([128, NHL, 512], BF16, name=f"at{pt}",
                                     tag=f"atq{pt}") for pt in range(4, ST)]
            # zero the left-of-diagonal strip each tile has at its FIRST use
            # (the mask multiply reads it; uninitialized SBUF can hold NaN
            # bit patterns and 0*NaN = NaN)
            for pt in range(4, ST):
                lo0 = 128 * (pt % 4)
                if lo0 > 0:
                    nc.gpsimd.memset(atq[pt][:, :, 0:lo0], 0.0)

            def emit_wo_block(st, split_dma=False):
                ob = pback.tile([128, D], F16, name=f"ob{st}",
                                tag="ob", bufs=2)
                for ncc in range(NC_):
                    wo_ps = ps.tile([128, 512], F32,
                                    name=f"wops{st}_{ncc}", tag="B", bufs=2)
                    for h in range(NHL):
                        nc.tensor.matmul(
                            wo_ps,
                            ot[h][:, st * 128:(st + 1) * 128],
                            wo[:, h, ncc * 512:(ncc + 1) * 512],
                            start=(h == 0), stop=(h == NHL - 1))
                    osl = slice(ncc * 512, (ncc + 1) * 512)
                    if ncc == 0:
                        nc.vector.tensor_copy(ob[:, osl], wo_ps)
                    else:
                        nc.scalar.copy(ob[:, osl], wo_ps)
                    if split_dma:
                        nc.sync.dma_start(
                            out=dOUT[st * 128:(st + 1) * 128, osl],
                            in_=ob[:, osl])
                if not split_dma:
                    nc.sync.dma_start(out=dOUT[st * 128:(st + 1) * 128, :],
                                      in_=ob)

            def dn4(qc, pt, dn_ps, npt):
                lo = max(0, pt * 128 - qc * 512)
                for h in range(NHL):
                    nc.tensor.matmul(dn_ps[32 * h:32 * h + 1, lo:512], ones_bf,
                                     atq[pt][:, h, lo:512],
                                     start=(pt == 0), stop=(pt == npt - 1),
                                     tile_position=(0, 32 * h))

            for qc in range(SK):
                npt = qc * 4 + 4
                qsl = slice(qc * 512, (qc + 1) * 512)
                dn_ps = ps.tile([128, 512], F32, name=f"dn{qc}", tag="dn",
                                bufs=1)
                # wo blocks of the previous q-chunk, spread across pt-steps
                wo_blocks = list(range((qc - 1) * 4, qc * 4)) if qc > 0 else []
                # scores (2 heads batched per 2-bank PSUM tile) + exp
                for pt in range(npt):
                    if qc > 0 and not (qc == 1 and pt >= 4):
                        scores_pt(qc, pt, ps, 2, atq[pt])
                    if pt >= 1:
                        dn4(qc, pt - 1, dn_ps, npt)
                    if wo_blocks and pt % max(1, npt // 4) == max(1, npt // 4) - 1:
                        emit_wo_block(wo_blocks.pop(0))
                dn4(qc, npt - 1, dn_ps, npt)
                while wo_blocks:
                    emit_wo_block(wo_blocks.pop(0))
                # 1/dn = exp(-ln(dn)) on ACT
                lnd = pback.tile([128, 512], F32, name=f"lnd{qc}", tag="lnd",
                                 bufs=2)
                nc.scalar.activation(lnd, dn_ps,
                                     mybir.ActivationFunctionType.Ln)
                rc = pback.tile([128, 512], BF16, name=f"rc{qc}", tag="rc",
                                bufs=2)
                nc.scalar.activation(rc, lnd,
                                     mybir.ActivationFunctionType.Exp,
                                     scale=-1.0)
                # per-head broadcast of 1/dn via PE ones-matmul
                rdb = [pback.tile([128, 512], BF16, name=f"rdb{h}_{qc}",
                                  tag=f"rdb{h}", bufs=2) for h in range(NHL)]
                for h in range(NHL):
                    bc_ps = ps.tile([128, 512], F32, name=f"bc{h}_{qc}",
                                    tag="dn", bufs=1)
                    nc.tensor.matmul(bc_ps, onesr_bf[32 * h:32 * h + 1, :],
                                     rc[32 * h:32 * h + 1, :],
                                     start=True, stop=True,
                                     tile_position=(32 * h, 0))
                    if h % 2 == 0:
                        nc.vector.tensor_copy(rdb[h], bc_ps)
                    else:
                        nc.scalar.copy(rdb[h], bc_ps)
                if qc == 0:
                    # fill the qc0 recip-chain PE bubble with qc1's fresh
                    # diagonal tiles (pts 4-7 are not read by round 0)
                    for pt in range(4, 8):
                        scores_pt(1, pt, ps, 2, atq[pt])
                if debug:
                    nc.gpsimd.dma_start(out=dDN[:, qc, :], in_=lnd)
                # AV + normalize
                for h in range(NHL):
                    o_ps = ps.tile([128, 512], F32, name=f"ops{h}{qc}",
                                   tag="C", bufs=1)
                    for pt in range(npt):
                        lo = max(0, pt * 128 - qc * 512)
                        nc.tensor.matmul(o_ps[:, lo:512], vv[h][:, pt, :],
                                         atq[pt][:, h, lo:512],
                                         start=(pt == 0), stop=(pt == npt - 1))
                    if debug and h == 0:
                        dbg_o = pback.tile([128, 512], F32, name=f"dbgo{qc}",
                                           tag="dbgo", bufs=2)
                        nc.vector.tensor_copy(dbg_o, o_ps)
                        nc.gpsimd.dma_start(out=dOPS[:, qsl], in_=dbg_o)
                        nc.gpsimd.dma_start(out=dRDB[:, qc, :], in_=rdb[0])
                        for pt in range(npt):
                            nc.gpsimd.dma_start(out=dATQ[:, pt, :],
                                                in_=atq[pt][:, 0, :])
                    nc.vector.tensor_mul(ot[h][:, qsl], o_ps, rdb[h])
            for st in range((SK - 1) * 4, SK * 4):
                emit_wo_block(st, split_dma=True)
            if debug:
                nc.gpsimd.dma_start(out=dQR[:, :], in_=qr[0])
                nc.gpsimd.dma_start(out=dVV[:, :, :], in_=vv[0])
                nc.gpsimd.dma_start(out=dOT[:, :], in_=ot[0])

    return nc


# ======================= host-side preparation ===========================

def host_prep(X, Wq_w, Wq_b, Wo_w, Wo_b, rms_w, n_cores=8, NHL=4):
    B, S, D = X.shape
    DK = 128
    c = DK ** -0.25
    inv = 1.0 / (ROPE_BASE ** (np.arange(0, DK, 2, dtype=np.float64) / DK))
    ang = np.arange(S, dtype=np.float64)[:, None] * inv[None, :]
    cos = np.concatenate([np.cos(ang), np.cos(ang)], -1)     # (S, DK)
    sin = np.concatenate([np.sin(ang), np.sin(ang)], -1)
    COSA = (cos.T * c).astype(np.float16)                    # (DK, S)
    SINT = (sin.T * c).astype(np.float32)
    SINA = np.concatenate([-SINT[:DK // 2], SINT[DK // 2:]], 0).astype(np.float16)
    tri = np.triu(np.ones((128, 128), np.float32))            # keep q >= p
    MASKD = np.zeros((128, 4, 512), np.float32)
    for v in range(4):
        lo = v * 128
        MASKD[:, v, lo:lo + 128] = tri
        MASKD[:, v, lo + 128:] = 1.0
    MASKD = MASKD.astype(ml_dtypes.bfloat16)

    Wq_eff = (Wq_w * rms_w[None, :]).astype(np.float32)       # fold rms weight
    in_maps = []
    groups = n_cores // B
    ML = NHL * DK
    S = X.shape[1]
    SK, KT = S // 512, D // 128
    for core in range(n_cores):
        b = core // groups
        hg = core % groups
        msl = slice(hg * ML, (hg + 1) * ML)
        XTf = X[b].T.astype(np.float16)                                   # (D, S)
        # device layout [p, c, k, s]: row d = k*128+p, chunk c of 512
        XT = np.ascontiguousarray(
            XTf.reshape(KT, 128, SK, 512).transpose(1, 2, 0, 3))
        WQf = Wq_eff[msl, :].T.astype(np.float16)                         # (D, ML)
        WQT = np.ascontiguousarray(
            WQf.reshape(KT, 128, NHL, 128).transpose(1, 2, 0, 3))
        WOf = Wo_w[:, msl].T.astype(np.float16)                           # (ML, D)
        WOT = np.ascontiguousarray(WOf.reshape(NHL, 128, D).transpose(1, 0, 2))
        QB = np.ascontiguousarray(
            Wq_b[msl].reshape(NHL, 128).T).astype(np.float32)             # (128, NHL)
        in_maps.append({
            "XT": XT, "WQT": WQT, "WOT": WOT, "QB": QB,
            "COSA": COSA, "SINA": SINA, "MASKD": MASKD,
        })
    return in_maps


def host_reduce(X, Wo_b, results, n_cores=8):
    B, S, D = X.shape
    groups = n_cores // B
    out = np.empty((B, S, D), np.float32)
    for b in range(B):
        acc = X[b].astype(np.float32).copy()
        for hg in range(groups):
            acc += results[b * groups + hg]["OUTP"].astype(np.float32)
        acc += Wo_b[None, :]
        out[b] = acc
    return out


# ======================= public entry point ==============================

_CACHE = {}


def _get_nc():
    if "nc" not in _CACHE:
        nc = build_core(S=2048, D=2048, NHL=4, DK=128, SHIFT=10.0)
        legalize_sync_waits(nc, max_waits=1)
        _CACHE["nc"] = nc
    return _CACHE["nc"]


def kernel(X, Wq_w, Wq_b, Wo_w, Wo_b, rms_w):
    """Full-input MHA block: returns X + MHA(RMSNorm(X)) as np.float32."""
    from concourse.bass_utils import run_bass_kernel_spmd

    X = np.asarray(X, np.float32)
    Wq_w = np.asarray(Wq_w, np.float32)
    Wq_b = np.asarray(Wq_b, np.float32)
    Wo_w = np.asarray(Wo_w, np.float32)
    Wo_b = np.asarray(Wo_b, np.float32)
    rms_w = np.asarray(rms_w, np.float32)

    nc = _get_nc()
    in_maps = host_prep(X, Wq_w, Wq_b, Wo_w, Wo_b, rms_w)
    res = run_bass_kernel_spmd(nc, in_maps, core_ids=list(range(8)))
    return host_reduce(X, Wo_b, res.results)
